# revision 19
# baseline (speedup 1.0000x reference)
"""TRN2 Bass kernel for nn_CausalSelfAttention_4054449128214.

The reference returns out_s + stop_gradient(out_full - out_s), whose forward
value is exactly out_full -- plain dense causal self-attention. So the kernel
computes: qkv = x@W_attn (+b), per-head causal softmax attention, y@W_proj (+b).

Sharding (8 cores, no collectives): Megatron head-parallel as the baseline --
cores 0-3 own head pairs (0,1)..(6,7); cores 4-7 own heads 8..11 duplicated
for SPMD shape-uniformity (dup W_proj rows zeroed). Host sums the 8 partial
[C,T] projections (the row-parallel all-reduce) and adds b_proj.

v3 speed design:
 - fp8(e4m3) DoubleRow matmuls (0.5 cy/row): QK compute, V compute (directly
   transposed via stationary=x8^T blocks), scores (zero-B + broadcast moving),
   AV over key-chunk pairs, late projection.
 - Exact-ish path only where absmax-sensitive: queries [0,EARLY) use bf16 V
   (from bf16 x tokens) + Act-exp bf16 e + bf16 AV + bf16 proj.
 - exp: Act engine writes fp8 e directly for off-diagonal regions; diagonal
   128x128 triangles use Schraudolph exp (tensor_scalar int32 bitcast) +
   mask-multiply, which masks and casts to fp8 in one DVE op.
 - bf16/fp8 DMA with host-side layout; b_attn asserted zero, b_proj added on
   host after the partial sum. Measured CPU-sim rel err of this scheme: 1.1e-2.
"""

import numpy as np
import ml_dtypes

import concourse.bacc as bacc
import concourse.mybir as mybir
import concourse.tile as tile
from concourse.bass_utils import run_bass_kernel_spmd

F32 = mybir.dt.float32
F32R = mybir.dt.float32r
BF16 = mybir.dt.bfloat16
F8 = mybir.dt.float8e4
I32 = mybir.dt.int32
NPF8 = ml_dtypes.float8_e4m3
NPBF16 = ml_dtypes.bfloat16
DR = mybir.MatmulPerfMode.DoubleRow
AF = mybir.ActivationFunctionType
MUL = mybir.AluOpType.mult

T = 1024
C = 768
NH = 12
HS = 64
NCORES = 8
NCC = 6            # 128-row contraction chunks of C
EARLY = 128        # queries [0,EARLY) take the exact-ish bf16 path
SCALE = 0.125      # 1/sqrt(HS)
# Schraudolph exp-approx constants (truncating int32 convert), scale folded.
SCH_A = 12102203.161561485 * SCALE
SCH_B = 1065353216.0 - 366392.0

HEAD_MAP = [(0, 1), (2, 3), (4, 5), (6, 7), (8, 8), (9, 9), (10, 10), (11, 11)]

_CACHE: dict = {}


def _build_program():
    nc = bacc.Bacc("TRN2", target_bir_lowering=False, debug=False,
                   num_devices=NCORES)
    # DRAM I/O -- host lays these out exactly as the SBUF tiles.
    x8d = nc.dram_tensor("x8", [128, NCC * T], F8, kind="ExternalInput").ap()
    xbd = nc.dram_tensor("xb", [128, NCC * EARLY], BF16, kind="ExternalInput").ap()
    # fp8 pack: wq [3,2,128] | wk [3,2,128] | wv [3,2,128] | wp8 [6,2,128]
    w8d = nc.dram_tensor("w8", [128, 3 * 768 + 1536], F8, kind="ExternalInput").ap()
    # bf16 pack: wvb [6,128] | wpb [6,128]
    wbd = nc.dram_tensor("wb", [128, NCC * 256], BF16, kind="ExternalInput").ap()
    outd = nc.dram_tensor("outT", [128, NCC * T], BF16, kind="ExternalOutput").ap()

    with tile.TileContext(nc) as tc:
        with (
            tc.tile_pool(name="cst", bufs=1) as cp,
            tc.tile_pool(name="pa", bufs=1, space="PSUM") as pa,    # qp/vp, kp
            tc.tile_pool(name="pc", bufs=2, space="PSUM") as pc,    # sc pairs + pm pairs
            tc.tile_pool(name="pd", bufs=2, space="PSUM") as pd,    # po
        ):
            # ---------------- SBUF tiles + input DMA ----------------
            x8 = cp.tile([128, NCC, T], F8, tag="x8")
            xb = cp.tile([128, NCC, EARLY], BF16, tag="xb")
            wq8 = cp.tile([128, 3, 2, 128], F8, tag="wq8")
            wk8 = cp.tile([128, 3, 2, 128], F8, tag="wk8")
            wv8 = cp.tile([128, 3, 2, 128], F8, tag="wv8")
            wp8 = cp.tile([128, NCC, 2, 128], F8, tag="wp8")  # rows 0:64 used
            wvb = cp.tile([128, NCC, 128], BF16, tag="wvb")
            wpb = cp.tile([128, NCC, 128], BF16, tag="wpb")

            for j in range(3):   # x8 split by cc-pair so QK starts early
                nc.sync.dma_start(
                    out=x8[:, 2 * j:2 * j + 2, :],
                    in_=x8d.rearrange("p (c t) -> p c t", c=NCC)[:, 2 * j:2 * j + 2])
            nc.sync.dma_start(
                out=wq8[:].rearrange("p a b m -> p (a b m)"), in_=w8d[:, 0:768])
            nc.sync.dma_start(
                out=wk8[:].rearrange("p a b m -> p (a b m)"), in_=w8d[:, 768:1536])
            nc.sync.dma_start(
                out=wv8[:].rearrange("p a b m -> p (a b m)"), in_=w8d[:, 1536:2304])
            nc.sync.dma_start(
                out=wp8[:].rearrange("p a b m -> p (a b m)"), in_=w8d[:, 2304:3840])
            nc.gpsimd.dma_start(
                out=xb[:].rearrange("p c t -> p (c t)"), in_=xbd)
            nc.gpsimd.dma_start(
                out=wvb[:].rearrange("p c m -> p (c m)"), in_=wbd[:, 0:768])
            nc.gpsimd.dma_start(
                out=wpb[:].rearrange("p c m -> p (c m)"), in_=wbd[:, 768:1536])

            # preload the Exp activation table during the DMA window
            dummy = cp.tile([1, 2], F32, tag="dummy")
            nc.gpsimd.memset(dummy[:], 0.0)
            nc.scalar.activation(dummy[:], dummy[:], AF.Exp, scale=1.0)

            q8 = cp.tile([128, T], F8, tag="q8")
            # k8[h-rows, kc, 0, m]; B-half [..,1,..] stays zero (scores DR trick)
            k8 = cp.tile([128, 8, 2, 128], F8, tag="k8")
            nc.vector.memset(k8[:, :, 1, :], 0.0)
            # per-head vaug: [keys-in-chunk, pair, half, 65]; col 64 = ones
            # 128 cols per half: 64 v-dims | ones col 64 | zero pad. Full-128
            # halves keep the legalizer's standalone dual-fp8 Ldweights ISA-
            # valid; padding costs nothing (matmul time = moving cols).
            vaug8 = [cp.tile([128, 4, 2, 128], F8, tag=f"vaug8_{h}",
                             name=f"vaug8_{h}") for h in range(2)]
            vauga = [cp.tile([128, 128], BF16, tag=f"vauga_{h}",
                             name=f"vauga_{h}") for h in range(2)]
            # cols 64:128 are all-ones: the AV matmul then replicates the
            # softmax denominator into po rows 64:127, so the reciprocal can
            # run on 64 partitions directly -- no partition_broadcast needed.
            for h in range(2):
                nc.gpsimd.memset(vaug8[h][:, :, :, 0:64], 0.0)
                nc.gpsimd.memset(vaug8[h][:, :, :, 64:128], 1.0)
                nc.gpsimd.memset(vauga[h][:, 0:64], 0.0)
                nc.gpsimd.memset(vauga[h][:, 64:128], 1.0)

            # e mega-tiles [keys, head, pair-half, 512] per (qt, pair).
            # Only 3 sub-diagonal strips are ever read-before-write -> memset
            # just those (they are static and never re-dirtied).
            e8 = {}
            for qt in range(2):
                for p in range(2 * qt + 2):
                    tl_ = cp.tile([128, 2, 2, 512], F8, tag=f"e8_{qt}{p}",
                                  name=f"e8_{qt}{p}")
                    e8[(qt, p)] = tl_
            for (qt, p, half, lo) in ((0, 1, 1, 256), (1, 2, 1, 0), (1, 3, 1, 256)):
                nc.gpsimd.memset(e8[(qt, p)][:, :, half, lo:lo + 128], 0.0)
            eam = cp.tile([128, 2, EARLY], BF16, tag="eam")

            y8 = cp.tile([64, 2, T - EARLY], F8, tag="y8")
            ya = cp.tile([128, EARLY], BF16, tag="ya")
            outs = cp.tile([128, NCC, T], BF16, tag="outs")

            # ---------------- QKV ----------------
            def emit_qk(tt):  # tt: T-half index
                ts_ = slice(tt * 512, tt * 512 + 512)
                qp = pa.tile([128, 512], F32, tag="qp")
                kp = pa.tile([128, 512], F32, tag="kp")
                for j in range(3):
                    nc.tensor.matmul(qp[:], wq8[:, j], x8[:, 2 * j:2 * j + 2, ts_],
                                     start=(j == 0), stop=(j == 2), perf_mode=DR)
                for j in range(3):
                    nc.tensor.matmul(kp[:], wk8[:, j], x8[:, 2 * j:2 * j + 2, ts_],
                                     start=(j == 0), stop=(j == 2), perf_mode=DR)
                nc.vector.tensor_copy(q8[:, ts_], qp[:])
                nc.vector.tensor_copy(
                    k8[:, 4 * tt:4 * tt + 4, 0, :],
                    kp[:].rearrange("p (kc m) -> p kc m", kc=4))

            def emit_vt():
                for half in range(2):  # 4 key-blocks per psum tile
                    vpt = pa.tile([128, 512], F32, tag="qp")
                    vp = vpt[:].rearrange("p (b m) -> p b m", b=4)
                    for b in range(4):
                        kb = half * 4 + b
                        tb = slice(kb * 128, kb * 128 + 128)
                        for j in range(3):
                            nc.tensor.matmul(
                                vp[:, b], x8[:, 2 * j:2 * j + 2, tb], wv8[:, j],
                                start=(j == 0), stop=(j == 2), perf_mode=DR)
                    # blocks (2half.., b) -> vaug8[h][:, pair, phalf, 0:64]
                    nc.vector.tensor_copy(
                        vaug8[0][:, 2 * half:2 * half + 2, :, 0:64]
                        .rearrange("p pr hf d -> p (pr hf) d"),
                        vp[:, :, 0:64])
                    nc.scalar.activation(
                        vaug8[1][:, 2 * half:2 * half + 2, :, 0:64]
                        .rearrange("p pr hf d -> p (pr hf) d"),
                        vp[:, :, 64:128], AF.Copy)
                # exact bf16 V for keys [0, EARLY)
                vpat = pa.tile([128, 512], F32, tag="qp")
                for cc in range(NCC):
                    nc.tensor.matmul(vpat[:, 0:128], xb[:, cc, :], wvb[:, cc, :],
                                     start=(cc == 0), stop=(cc == NCC - 1))
                for h in range(2):
                    nc.vector.tensor_copy(vauga[h][:, 0:64],
                                          vpat[:, h * 64:h * 64 + 64])

            # ---------------- attention ----------------
            schi = [cp.tile([128, 2, 512], I32, tag=f"schi{i}", name=f"schi{i}")
                    for i in range(4)]
            sch_n = [0]

            def emit_exp(sc_, dst, lo, use_schr):
                # sc_: [128, 512] psum view; dst: e8 target slice from lo
                if not use_schr:
                    nc.scalar.activation(dst, sc_[lo:512] if False else
                                         sc_[:, lo:512], AF.Exp, scale=SCALE)
                else:
                    si = schi[sch_n[0] % 4]
                    sch_n[0] += 1
                    nc.vector.tensor_scalar(
                        si[:, 0, lo:512], sc_[:, lo:512], SCH_A, SCH_B,
                        MUL, mybir.AluOpType.add)
                    nc.gpsimd.tensor_copy(dst, si[:, 0, lo:512].bitcast(F32))

            pos_store = {}

            def attn_scores(qt):
                qs = slice(qt * 512, qt * 512 + 512)
                npair = 2 * qt + 2
                for p in range(npair):
                    et = e8[(qt, p)]
                    for h in range(2):
                        hrow = slice(h * 64, h * 64 + 64)
                        scp = pc.tile([128, 2, 512], F32, tag="sc")
                        rags = []
                        for half in range(2):
                            kc = 2 * p + half
                            kcr = kc - 4 * qt  # >=0 on diagonal chunks
                            rag = max(0, kcr) * 128
                            elo = 0 if (qt == 0 and kc == 0) else rag
                            rags.append((kc, kcr, rag, elo))
                            nc.tensor.matmul(
                                scp[:, half, elo:512],
                                k8[hrow, kc], q8[hrow, qs].unsqueeze(1)
                                .broadcast_to([64, 2, 512])[:, :, elo:512],
                                start=True, stop=True, perf_mode=DR)
                        kc0sp = (qt == 0 and p == 0)
                        if kc0sp:
                            nc.scalar.activation(
                                eam[:, h, :], scp[:, 0, 0:EARLY],
                                AF.Exp, scale=SCALE)
                        use_schr = (qt == 1 and h == 1)
                        if rags[0][2] == rags[1][2] and not kc0sp:
                            # same ragged start: one batched exp over both
                            # halves (off-diagonal pairs)
                            rag = rags[0][2]
                            if use_schr:
                                si = schi[sch_n[0] % 4]
                                sch_n[0] += 1
                                nc.vector.tensor_scalar(
                                    si[:, :, rag:512],
                                    scp[:, :, rag:512], SCH_A, SCH_B,
                                    MUL, mybir.AluOpType.add)
                                nc.gpsimd.tensor_copy(
                                    et[:, h, :, rag:512],
                                    si[:, :, rag:512].bitcast(F32))
                            else:
                                nc.scalar.activation(
                                    et[:, h, :, rag:512], scp[:, :, rag:512],
                                    AF.Exp, scale=SCALE)
                        else:
                            for half in range(2):
                                kc, kcr, rag, elo = rags[half]
                                lo = EARLY if (qt == 0 and kc == 0) else rag
                                emit_exp(scp[:, half, :],
                                         et[:, h, half, lo:512], lo,
                                         use_schr)
                    # causal triangle masks, batched across heads (Pool)
                    for half in range(2):
                        kc = 2 * p + half
                        kcr = kc - 4 * qt
                        if kcr < 0 or (qt == 0 and kc == 0):
                            continue
                        rag = kcr * 128
                        nc.gpsimd.affine_select(
                            et[:, :, half, rag:rag + 128],
                            et[:, :, half, rag:rag + 128],
                            pattern=[[0, 2], [1, 128]],
                            compare_op=mybir.AluOpType.is_ge, fill=0.0,
                            base=0, channel_multiplier=-1)
                if qt == 0:
                    nc.gpsimd.affine_select(
                        eam[:], eam[:], pattern=[[0, 2], [1, EARLY]],
                        compare_op=mybir.AluOpType.is_ge, fill=0.0,
                        base=0, channel_multiplier=-1)

            def attn_av(qt):
                npair = 2 * qt + 2
                pos = [pd.tile([128, 512], F32, tag="po", name=f"po{qt}{h}")
                       for h in range(2)]
                pos_store[qt] = pos
                for p in range(npair):
                    et = e8[(qt, p)]
                    astart = max(0, 256 * p - 512 * qt)
                    if qt == 0 and p == 0:
                        astart = EARLY
                    for h in range(2):
                        nc.tensor.matmul(
                            pos[h][:, astart:512], vaug8[h][:, p, :, :],
                            et[:, h, :, astart:512],
                            start=(p == 0), stop=(p == npair - 1),
                            perf_mode=DR)
                if qt == 0:
                    for h in range(2):
                        nc.tensor.matmul(pos[h][:, 0:EARLY], vauga[h][:],
                                         eam[:, h, :], start=True, stop=True)

            def attn_y(qt):
                pos = pos_store[qt]
                for h in range(2):
                    hrow = slice(h * 64, h * 64 + 64)
                    po = pos[h]
                    rbb = cp.tile([64, 512], F32, tag=f"rbb{h}", name=f"rbb{h}")
                    nc.vector.reciprocal(rbb[:], po[64:128, :])
                    if qt == 0:
                        nc.vector.tensor_tensor(
                            ya[hrow, :], po[0:64, 0:EARLY], rbb[:, 0:EARLY], MUL)
                        nc.vector.tensor_tensor(
                            y8[:, h, 0:512 - EARLY], po[0:64, EARLY:512],
                            rbb[:, EARLY:512], MUL)
                    else:
                        nc.vector.tensor_tensor(
                            y8[:, h, 512 - EARLY:1024 - EARLY], po[0:64, :],
                            rbb[:], MUL)

            # ---------------- projection ----------------
            def emit_proj(qt):
                for cp_ in range(3):  # chunk pairs (cc, cc+1)
                    pm = pc.tile([128, 2, 512], F32, tag="sc")
                    for i in range(2):
                        cc = 2 * cp_ + i
                        if qt == 0:
                            nc.tensor.matmul(pm[:, i, 0:EARLY], wpb[:, cc, :],
                                             ya[:], start=True, stop=True)
                            nc.tensor.matmul(
                                pm[:, i, EARLY:512], wp8[0:64, cc],
                                y8[:, :, 0:512 - EARLY],
                                start=True, stop=True, perf_mode=DR)
                        else:
                            nc.tensor.matmul(
                                pm[:, i, :], wp8[0:64, cc],
                                y8[:, :, 512 - EARLY:1024 - EARLY],
                                start=True, stop=True, perf_mode=DR)
                    dst = outs[:, 2 * cp_:2 * cp_ + 2, qt * 512:qt * 512 + 512]
                    if cp_ % 2 == 0:
                        nc.vector.tensor_copy(dst, pm[:])
                    else:
                        nc.scalar.activation(dst, pm[:], AF.Copy)
                    if cp_ == 1:  # first 4 chunks staged: start half the DMA
                        nc.sync.dma_start(
                            out=outd.rearrange("p (c t) -> p c t", c=NCC)
                            [:, 0:4, qt * 512:qt * 512 + 512],
                            in_=outs[:, 0:4, qt * 512:qt * 512 + 512])
                nc.sync.dma_start(
                    out=outd.rearrange("p (c t) -> p c t", c=NCC)
                    [:, 4:6, qt * 512:qt * 512 + 512],
                    in_=outs[:, 4:6, qt * 512:qt * 512 + 512])

            # ---------------- schedule ----------------
            emit_qk(0)
            emit_qk(1)
            emit_vt()
            attn_scores(0)
            attn_av(0)
            attn_y(0)
            attn_scores(1)   # PE chews qt1 scores while DVE normalizes qt0
            emit_proj(0)
            attn_av(1)
            attn_y(1)
            emit_proj(1)
    nc.compile()
    return nc


def _in_maps(x, W_attn, b_attn, W_proj, b_proj):
    xT = x.reshape(T, C).T  # [C, T] f32
    x8 = np.ascontiguousarray(
        xT.reshape(NCC, 128, T).transpose(1, 0, 2).reshape(128, NCC * T)
    ).astype(NPF8)
    xbv = np.ascontiguousarray(
        xT[:, 0:EARLY].reshape(NCC, 128, EARLY).transpose(1, 0, 2)
        .reshape(128, NCC * EARLY)).astype(NPBF16)
    maps = []
    for core in range(NCORES):
        h0, h1 = HEAD_MAP[core]
        qcols = (list(range(h0 * HS, (h0 + 1) * HS))
                 + list(range(h1 * HS, (h1 + 1) * HS)))
        kcols = [C + c for c in qcols]
        vcols = [2 * C + c for c in qcols]

        def drpack(Wsl):  # [768, 128] -> [128, 3*2*128] DR-stationary layout
            return np.ascontiguousarray(
                Wsl.reshape(3, 2, 128, 128).transpose(2, 0, 1, 3)
                .reshape(128, 3 * 2 * 128))

        wq = drpack(W_attn[:, qcols])
        wk = drpack(W_attn[:, kcols])
        wv = drpack(W_attn[:, vcols])
        wp_h0 = W_proj[h0 * HS:(h0 + 1) * HS, :]                     # [64, 768]
        wp_h1 = (np.zeros_like(wp_h0) if h1 == h0
                 else W_proj[h1 * HS:(h1 + 1) * HS, :])
        wp8 = np.zeros((128, NCC, 2, 128), np.float32)
        wp8[0:64, :, 0, :] = wp_h0.reshape(64, NCC, 128)
        wp8[0:64, :, 1, :] = wp_h1.reshape(64, NCC, 128)
        w8 = np.concatenate(
            [wq, wk, wv, wp8.reshape(128, NCC * 2 * 128)], axis=1).astype(NPF8)
        wvb = np.ascontiguousarray(
            W_attn[:, vcols].reshape(NCC, 128, 128).transpose(1, 0, 2)
            .reshape(128, NCC * 128))
        wp_rows = np.concatenate([wp_h0, wp_h1], axis=0)             # [128, 768]
        wpb = np.ascontiguousarray(wp_rows.reshape(128, NCC * 128))
        wb = np.concatenate([wvb, wpb], axis=1).astype(NPBF16)
        maps.append({"x8": x8, "xb": xbv, "w8": w8, "wb": wb})
    return maps


def kernel(x, W_attn, b_attn, W_proj, b_proj, _trace=False, _trace_kwargs=None):
    x = np.asarray(x, np.float32)
    W_attn = np.asarray(W_attn, np.float32)
    b_attn = np.asarray(b_attn, np.float32)
    W_proj = np.asarray(W_proj, np.float32)
    b_proj = np.asarray(b_proj, np.float32)
    assert not np.any(b_attn), "kernel compiled for zero b_attn"

    if "nc" not in _CACHE:
        _CACHE["nc"] = _build_program()
    nc = _CACHE["nc"]

    maps = _in_maps(x, W_attn, b_attn, W_proj, b_proj)
    kw = {}
    if _trace:
        kw = dict(trace=True, **(_trace_kwargs or {}))
    br = run_bass_kernel_spmd(nc, maps, list(range(NCORES)), **kw)
    acc = np.zeros((C, T), np.float64)
    for core in range(NCORES):
        o = br.results[core]["outT"].astype(np.float64)  # [128, 6*T]
        acc += o.reshape(128, NCC, T).transpose(1, 0, 2).reshape(C, T)
    acc += b_proj[:, None]
    out = np.ascontiguousarray(acc.T.astype(np.float32)).reshape(1, T, C)
    _CACHE["last_results"] = br
    return out


# revision 21
# speedup vs baseline: 1.0020x; 1.0020x over previous
"""TRN2 Bass kernel for nn_CausalSelfAttention_4054449128214.

The reference returns out_s + stop_gradient(out_full - out_s), whose forward
value is exactly out_full -- plain dense causal self-attention. So the kernel
computes: qkv = x@W_attn (+b), per-head causal softmax attention, y@W_proj (+b).

Sharding (8 cores, no collectives): Megatron head-parallel as the baseline --
cores 0-3 own head pairs (0,1)..(6,7); cores 4-7 own heads 8..11 duplicated
for SPMD shape-uniformity (dup W_proj rows zeroed). Host sums the 8 partial
[C,T] projections (the row-parallel all-reduce) and adds b_proj.

v3 speed design:
 - fp8(e4m3) DoubleRow matmuls (0.5 cy/row): QK compute, V compute (directly
   transposed via stationary=x8^T blocks), scores (zero-B + broadcast moving),
   AV over key-chunk pairs, late projection.
 - Exact-ish path only where absmax-sensitive: queries [0,EARLY) use bf16 V
   (from bf16 x tokens) + Act-exp bf16 e + bf16 AV + bf16 proj.
 - exp: Act engine writes fp8 e directly for off-diagonal regions; diagonal
   128x128 triangles use Schraudolph exp (tensor_scalar int32 bitcast) +
   mask-multiply, which masks and casts to fp8 in one DVE op.
 - bf16/fp8 DMA with host-side layout; b_attn asserted zero, b_proj added on
   host after the partial sum. Measured CPU-sim rel err of this scheme: 1.1e-2.
"""

import numpy as np
import ml_dtypes

import concourse.bacc as bacc
import concourse.mybir as mybir
import concourse.tile as tile
from concourse.bass_utils import run_bass_kernel_spmd

F32 = mybir.dt.float32
F32R = mybir.dt.float32r
BF16 = mybir.dt.bfloat16
F8 = mybir.dt.float8e4
I32 = mybir.dt.int32
NPF8 = ml_dtypes.float8_e4m3
NPBF16 = ml_dtypes.bfloat16
DR = mybir.MatmulPerfMode.DoubleRow
AF = mybir.ActivationFunctionType
MUL = mybir.AluOpType.mult

T = 1024
C = 768
NH = 12
HS = 64
NCORES = 8
NCC = 6            # 128-row contraction chunks of C
EARLY = 128        # queries [0,EARLY) take the exact-ish bf16 path
SCALE = 0.125      # 1/sqrt(HS)
# Schraudolph exp-approx constants (truncating int32 convert), scale folded.
SCH_A = 12102203.161561485 * SCALE
SCH_B = 1065353216.0 - 366392.0

HEAD_MAP = [(0, 1), (2, 3), (4, 5), (6, 7), (8, 8), (9, 9), (10, 10), (11, 11)]

_CACHE: dict = {}


def _build_program():
    nc = bacc.Bacc("TRN2", target_bir_lowering=False, debug=False,
                   num_devices=NCORES)
    # DRAM I/O -- host lays these out exactly as the SBUF tiles.
    x8d = nc.dram_tensor("x8", [128, NCC * T], F8, kind="ExternalInput").ap()
    xbd = nc.dram_tensor("xb", [128, NCC * EARLY], BF16, kind="ExternalInput").ap()
    # fp8 pack: wq [3,2,128] | wk [3,2,128] | wv [3,2,128] | wp8 [6,2,128]
    w8d = nc.dram_tensor("w8", [128, 3 * 768 + 1536], F8, kind="ExternalInput").ap()
    # bf16 pack: wvb [6,128] | wpb [6,128]
    wbd = nc.dram_tensor("wb", [128, NCC * 256], BF16, kind="ExternalInput").ap()
    outd = nc.dram_tensor("outT", [128, NCC * T], BF16, kind="ExternalOutput").ap()

    with tile.TileContext(nc) as tc:
        with (
            tc.tile_pool(name="cst", bufs=1) as cp,
            tc.tile_pool(name="pc", bufs=3, space="PSUM") as pc,    # all mm pairs
            tc.tile_pool(name="pd", bufs=2, space="PSUM") as pd,    # po
        ):
            # ---------------- SBUF tiles + input DMA ----------------
            x8 = cp.tile([128, NCC, T], F8, tag="x8")
            xb = cp.tile([128, NCC, EARLY], BF16, tag="xb")
            wq8 = cp.tile([128, 3, 2, 128], F8, tag="wq8")
            wk8 = cp.tile([128, 3, 2, 128], F8, tag="wk8")
            wv8 = cp.tile([128, 3, 2, 128], F8, tag="wv8")
            wp8 = cp.tile([128, NCC, 2, 128], F8, tag="wp8")  # rows 0:64 used
            wvb = cp.tile([128, NCC, 128], BF16, tag="wvb")
            wpb = cp.tile([128, NCC, 128], BF16, tag="wpb")

            for j in range(3):   # x8 split by cc-pair so QK starts early
                nc.sync.dma_start(
                    out=x8[:, 2 * j:2 * j + 2, :],
                    in_=x8d.rearrange("p (c t) -> p c t", c=NCC)[:, 2 * j:2 * j + 2])
            nc.sync.dma_start(
                out=wq8[:].rearrange("p a b m -> p (a b m)"), in_=w8d[:, 0:768])
            nc.sync.dma_start(
                out=wk8[:].rearrange("p a b m -> p (a b m)"), in_=w8d[:, 768:1536])
            nc.sync.dma_start(
                out=wv8[:].rearrange("p a b m -> p (a b m)"), in_=w8d[:, 1536:2304])
            nc.sync.dma_start(
                out=wp8[:].rearrange("p a b m -> p (a b m)"), in_=w8d[:, 2304:3840])
            nc.gpsimd.dma_start(
                out=xb[:].rearrange("p c t -> p (c t)"), in_=xbd)
            nc.gpsimd.dma_start(
                out=wvb[:].rearrange("p c m -> p (c m)"), in_=wbd[:, 0:768])
            nc.gpsimd.dma_start(
                out=wpb[:].rearrange("p c m -> p (c m)"), in_=wbd[:, 768:1536])

            # preload the Exp activation table during the DMA window
            dummy = cp.tile([1, 2], F32, tag="dummy")
            nc.gpsimd.memset(dummy[:], 0.0)
            nc.scalar.activation(dummy[:], dummy[:], AF.Exp, scale=1.0)

            q8 = cp.tile([128, T], F8, tag="q8")
            # k8[h-rows, kc, 0, m]; B-half [..,1,..] stays zero (scores DR trick)
            k8 = cp.tile([128, 8, 2, 128], F8, tag="k8")
            nc.vector.memset(k8[:, :, 1, :], 0.0)
            # per-head vaug: [keys-in-chunk, pair, half, 65]; col 64 = ones
            # 128 cols per half: 64 v-dims | ones col 64 | zero pad. Full-128
            # halves keep the legalizer's standalone dual-fp8 Ldweights ISA-
            # valid; padding costs nothing (matmul time = moving cols).
            vaug8 = [cp.tile([128, 4, 2, 128], F8, tag=f"vaug8_{h}",
                             name=f"vaug8_{h}") for h in range(2)]
            vauga = [cp.tile([128, 128], BF16, tag=f"vauga_{h}",
                             name=f"vauga_{h}") for h in range(2)]
            # cols 64:128 are all-ones: the AV matmul then replicates the
            # softmax denominator into po rows 64:127, so the reciprocal can
            # run on 64 partitions directly -- no partition_broadcast needed.
            for h in range(2):
                nc.gpsimd.memset(vaug8[h][:, :, :, 0:64], 0.0)
                nc.gpsimd.memset(vaug8[h][:, :, :, 64:128], 1.0)
                nc.gpsimd.memset(vauga[h][:, 0:64], 0.0)
                nc.gpsimd.memset(vauga[h][:, 64:128], 1.0)

            # e mega-tiles [keys, head, pair-half, 512] per (qt, pair).
            # Only 3 sub-diagonal strips are ever read-before-write -> memset
            # just those (they are static and never re-dirtied).
            e8 = {}
            for qt in range(2):
                for p in range(2 * qt + 2):
                    tl_ = cp.tile([128, 2, 2, 512], F8, tag=f"e8_{qt}{p}",
                                  name=f"e8_{qt}{p}")
                    e8[(qt, p)] = tl_
            for (qt, p, half, lo) in ((0, 1, 1, 256), (1, 2, 1, 0), (1, 3, 1, 256)):
                nc.gpsimd.memset(e8[(qt, p)][:, :, half, lo:lo + 128], 0.0)
            eam = cp.tile([128, 2, EARLY], BF16, tag="eam")

            y8 = cp.tile([64, 2, T - EARLY], F8, tag="y8")
            ya = cp.tile([128, EARLY], BF16, tag="ya")
            outs = cp.tile([128, NCC, T], BF16, tag="outs")

            # ---------------- QKV ----------------
            def emit_qk(tt):  # tt: T-half index
                ts_ = slice(tt * 512, tt * 512 + 512)
                qkp = pc.tile([128, 2, 512], F32, tag="sc")
                for j in range(3):
                    nc.tensor.matmul(qkp[:, 0, :], wq8[:, j],
                                     x8[:, 2 * j:2 * j + 2, ts_],
                                     start=(j == 0), stop=(j == 2), perf_mode=DR)
                for j in range(3):
                    nc.tensor.matmul(qkp[:, 1, :], wk8[:, j],
                                     x8[:, 2 * j:2 * j + 2, ts_],
                                     start=(j == 0), stop=(j == 2), perf_mode=DR)
                nc.vector.tensor_copy(q8[:, ts_], qkp[:, 0, :])
                nc.vector.tensor_copy(
                    k8[:, 4 * tt:4 * tt + 4, 0, :],
                    qkp[:, 1, :].rearrange("p (kc m) -> p kc m", kc=4))

            def emit_vt():
                vpt = pc.tile([128, 2, 512], F32, tag="sc")
                vp = vpt[:].rearrange("p a (k m) -> p (a k) m", k=4)
                for kb in range(8):
                    tb = slice(kb * 128, kb * 128 + 128)
                    for j in range(3):
                        nc.tensor.matmul(
                            vp[:, kb], x8[:, 2 * j:2 * j + 2, tb], wv8[:, j],
                            start=(j == 0), stop=(j == 2), perf_mode=DR)
                # blocks (kb) -> vaug8[h][:, pair, phalf, 0:64]
                for half in range(2):
                    nc.vector.tensor_copy(
                        vaug8[0][:, 2 * half:2 * half + 2, :, 0:64]
                        .rearrange("p pr hf d -> p (pr hf) d"),
                        vp[:, 4 * half:4 * half + 4, 0:64])
                    nc.scalar.activation(
                        vaug8[1][:, 2 * half:2 * half + 2, :, 0:64]
                        .rearrange("p pr hf d -> p (pr hf) d"),
                        vp[:, 4 * half:4 * half + 4, 64:128], AF.Copy)
                # exact bf16 V for keys [0, EARLY)
                vpat = pc.tile([128, 2, 512], F32, tag="sc")
                for cc in range(NCC):
                    nc.tensor.matmul(vpat[:, 0, 0:128], xb[:, cc, :],
                                     wvb[:, cc, :],
                                     start=(cc == 0), stop=(cc == NCC - 1))
                for h in range(2):
                    nc.vector.tensor_copy(vauga[h][:, 0:64],
                                          vpat[:, 0, h * 64:h * 64 + 64])

            # ---------------- attention ----------------
            schi = [cp.tile([128, 2, 512], I32, tag=f"schi{i}", name=f"schi{i}")
                    for i in range(4)]
            sch_n = [0]

            def emit_exp(sc_, dst, lo, use_schr):
                # sc_: [128, 512] psum view; dst: e8 target slice from lo
                if not use_schr:
                    nc.scalar.activation(dst, sc_[lo:512] if False else
                                         sc_[:, lo:512], AF.Exp, scale=SCALE)
                else:
                    si = schi[sch_n[0] % 4]
                    sch_n[0] += 1
                    nc.vector.tensor_scalar(
                        si[:, 0, lo:512], sc_[:, lo:512], SCH_A, SCH_B,
                        MUL, mybir.AluOpType.add)
                    nc.gpsimd.tensor_copy(dst, si[:, 0, lo:512].bitcast(F32))

            pos_store = {}

            def attn_scores(qt):
                qs = slice(qt * 512, qt * 512 + 512)
                npair = 2 * qt + 2
                for p in range(npair):
                    et = e8[(qt, p)]
                    for h in range(2):
                        hrow = slice(h * 64, h * 64 + 64)
                        scp = pc.tile([128, 2, 512], F32, tag="sc")
                        rags = []
                        for half in range(2):
                            kc = 2 * p + half
                            kcr = kc - 4 * qt  # >=0 on diagonal chunks
                            rag = max(0, kcr) * 128
                            elo = 0 if (qt == 0 and kc == 0) else rag
                            rags.append((kc, kcr, rag, elo))
                            nc.tensor.matmul(
                                scp[:, half, elo:512],
                                k8[hrow, kc], q8[hrow, qs].unsqueeze(1)
                                .broadcast_to([64, 2, 512])[:, :, elo:512],
                                start=True, stop=True, perf_mode=DR)
                        kc0sp = (qt == 0 and p == 0)
                        if kc0sp:
                            nc.scalar.activation(
                                eam[:, h, :], scp[:, 0, 0:EARLY],
                                AF.Exp, scale=SCALE)
                        use_schr = (qt == 1 and h == 1)
                        if rags[0][2] == rags[1][2] and not kc0sp:
                            # same ragged start: one batched exp over both
                            # halves (off-diagonal pairs)
                            rag = rags[0][2]
                            if use_schr:
                                si = schi[sch_n[0] % 4]
                                sch_n[0] += 1
                                nc.vector.tensor_scalar(
                                    si[:, :, rag:512],
                                    scp[:, :, rag:512], SCH_A, SCH_B,
                                    MUL, mybir.AluOpType.add)
                                nc.gpsimd.tensor_copy(
                                    et[:, h, :, rag:512],
                                    si[:, :, rag:512].bitcast(F32))
                            else:
                                nc.scalar.activation(
                                    et[:, h, :, rag:512], scp[:, :, rag:512],
                                    AF.Exp, scale=SCALE)
                        else:
                            for half in range(2):
                                kc, kcr, rag, elo = rags[half]
                                lo = EARLY if (qt == 0 and kc == 0) else rag
                                emit_exp(scp[:, half, :],
                                         et[:, h, half, lo:512], lo,
                                         use_schr)
                    # causal triangle masks, batched across heads (Pool)
                    for half in range(2):
                        kc = 2 * p + half
                        kcr = kc - 4 * qt
                        if kcr < 0 or (qt == 0 and kc == 0):
                            continue
                        rag = kcr * 128
                        nc.gpsimd.affine_select(
                            et[:, :, half, rag:rag + 128],
                            et[:, :, half, rag:rag + 128],
                            pattern=[[0, 2], [1, 128]],
                            compare_op=mybir.AluOpType.is_ge, fill=0.0,
                            base=0, channel_multiplier=-1)
                if qt == 0:
                    nc.gpsimd.affine_select(
                        eam[:], eam[:], pattern=[[0, 2], [1, EARLY]],
                        compare_op=mybir.AluOpType.is_ge, fill=0.0,
                        base=0, channel_multiplier=-1)

            def attn_av(qt):
                npair = 2 * qt + 2
                pos = [pd.tile([128, 512], F32, tag="po", name=f"po{qt}{h}")
                       for h in range(2)]
                pos_store[qt] = pos
                for p in range(npair):
                    et = e8[(qt, p)]
                    astart = max(0, 256 * p - 512 * qt)
                    if qt == 0 and p == 0:
                        astart = EARLY
                    for h in range(2):
                        nc.tensor.matmul(
                            pos[h][:, astart:512], vaug8[h][:, p, :, :],
                            et[:, h, :, astart:512],
                            start=(p == 0), stop=(p == npair - 1),
                            perf_mode=DR)
                if qt == 0:
                    for h in range(2):
                        nc.tensor.matmul(pos[h][:, 0:EARLY], vauga[h][:],
                                         eam[:, h, :], start=True, stop=True)

            def attn_y(qt):
                pos = pos_store[qt]
                for h in range(2):
                    hrow = slice(h * 64, h * 64 + 64)
                    po = pos[h]
                    rbb = cp.tile([64, 512], F32, tag=f"rbb{h}", name=f"rbb{h}")
                    nc.vector.reciprocal(rbb[:], po[64:128, :])
                    if qt == 0:
                        nc.vector.tensor_tensor(
                            ya[hrow, :], po[0:64, 0:EARLY], rbb[:, 0:EARLY], MUL)
                        nc.vector.tensor_tensor(
                            y8[:, h, 0:512 - EARLY], po[0:64, EARLY:512],
                            rbb[:, EARLY:512], MUL)
                    else:
                        nc.vector.tensor_tensor(
                            y8[:, h, 512 - EARLY:1024 - EARLY], po[0:64, :],
                            rbb[:], MUL)

            # ---------------- projection ----------------
            def emit_proj(qt):
                for cp_ in range(3):  # chunk pairs (cc, cc+1)
                    pm = pc.tile([128, 2, 512], F32, tag="sc")
                    for i in range(2):
                        cc = 2 * cp_ + i
                        if qt == 0:
                            nc.tensor.matmul(pm[:, i, 0:EARLY], wpb[:, cc, :],
                                             ya[:], start=True, stop=True)
                            nc.tensor.matmul(
                                pm[:, i, EARLY:512], wp8[0:64, cc],
                                y8[:, :, 0:512 - EARLY],
                                start=True, stop=True, perf_mode=DR)
                        else:
                            nc.tensor.matmul(
                                pm[:, i, :], wp8[0:64, cc],
                                y8[:, :, 512 - EARLY:1024 - EARLY],
                                start=True, stop=True, perf_mode=DR)
                    dst = outs[:, 2 * cp_:2 * cp_ + 2, qt * 512:qt * 512 + 512]
                    if cp_ == 0:
                        nc.vector.tensor_copy(dst, pm[:])
                    else:
                        nc.scalar.activation(dst, pm[:], AF.Copy)
                    if cp_ == 1:  # first 4 chunks staged: start half the DMA
                        nc.sync.dma_start(
                            out=outd.rearrange("p (c t) -> p c t", c=NCC)
                            [:, 0:4, qt * 512:qt * 512 + 512],
                            in_=outs[:, 0:4, qt * 512:qt * 512 + 512])
                nc.sync.dma_start(
                    out=outd.rearrange("p (c t) -> p c t", c=NCC)
                    [:, 4:6, qt * 512:qt * 512 + 512],
                    in_=outs[:, 4:6, qt * 512:qt * 512 + 512])

            # ---------------- schedule ----------------
            emit_qk(0)
            emit_qk(1)
            emit_vt()
            attn_scores(0)
            attn_av(0)
            attn_y(0)
            attn_scores(1)   # PE chews qt1 scores while DVE normalizes qt0
            emit_proj(0)
            attn_av(1)
            attn_y(1)
            emit_proj(1)
    nc.compile()
    return nc


def _in_maps(x, W_attn, b_attn, W_proj, b_proj):
    xT = x.reshape(T, C).T  # [C, T] f32
    x8 = np.ascontiguousarray(
        xT.reshape(NCC, 128, T).transpose(1, 0, 2).reshape(128, NCC * T)
    ).astype(NPF8)
    xbv = np.ascontiguousarray(
        xT[:, 0:EARLY].reshape(NCC, 128, EARLY).transpose(1, 0, 2)
        .reshape(128, NCC * EARLY)).astype(NPBF16)
    maps = []
    for core in range(NCORES):
        h0, h1 = HEAD_MAP[core]
        qcols = (list(range(h0 * HS, (h0 + 1) * HS))
                 + list(range(h1 * HS, (h1 + 1) * HS)))
        kcols = [C + c for c in qcols]
        vcols = [2 * C + c for c in qcols]

        def drpack(Wsl):  # [768, 128] -> [128, 3*2*128] DR-stationary layout
            return np.ascontiguousarray(
                Wsl.reshape(3, 2, 128, 128).transpose(2, 0, 1, 3)
                .reshape(128, 3 * 2 * 128))

        wq = drpack(W_attn[:, qcols])
        wk = drpack(W_attn[:, kcols])
        wv = drpack(W_attn[:, vcols])
        wp_h0 = W_proj[h0 * HS:(h0 + 1) * HS, :]                     # [64, 768]
        wp_h1 = (np.zeros_like(wp_h0) if h1 == h0
                 else W_proj[h1 * HS:(h1 + 1) * HS, :])
        wp8 = np.zeros((128, NCC, 2, 128), np.float32)
        wp8[0:64, :, 0, :] = wp_h0.reshape(64, NCC, 128)
        wp8[0:64, :, 1, :] = wp_h1.reshape(64, NCC, 128)
        w8 = np.concatenate(
            [wq, wk, wv, wp8.reshape(128, NCC * 2 * 128)], axis=1).astype(NPF8)
        wvb = np.ascontiguousarray(
            W_attn[:, vcols].reshape(NCC, 128, 128).transpose(1, 0, 2)
            .reshape(128, NCC * 128))
        wp_rows = np.concatenate([wp_h0, wp_h1], axis=0)             # [128, 768]
        wpb = np.ascontiguousarray(wp_rows.reshape(128, NCC * 128))
        wb = np.concatenate([wvb, wpb], axis=1).astype(NPBF16)
        maps.append({"x8": x8, "xb": xbv, "w8": w8, "wb": wb})
    return maps


def kernel(x, W_attn, b_attn, W_proj, b_proj, _trace=False, _trace_kwargs=None):
    x = np.asarray(x, np.float32)
    W_attn = np.asarray(W_attn, np.float32)
    b_attn = np.asarray(b_attn, np.float32)
    W_proj = np.asarray(W_proj, np.float32)
    b_proj = np.asarray(b_proj, np.float32)
    assert not np.any(b_attn), "kernel compiled for zero b_attn"

    if "nc" not in _CACHE:
        _CACHE["nc"] = _build_program()
    nc = _CACHE["nc"]

    maps = _in_maps(x, W_attn, b_attn, W_proj, b_proj)
    kw = {}
    if _trace:
        kw = dict(trace=True, **(_trace_kwargs or {}))
    br = run_bass_kernel_spmd(nc, maps, list(range(NCORES)), **kw)
    acc = np.zeros((C, T), np.float64)
    for core in range(NCORES):
        o = br.results[core]["outT"].astype(np.float64)  # [128, 6*T]
        acc += o.reshape(128, NCC, T).transpose(1, 0, 2).reshape(C, T)
    acc += b_proj[:, None]
    out = np.ascontiguousarray(acc.T.astype(np.float32)).reshape(1, T, C)
    _CACHE["last_results"] = br
    return out


# revision 22
# speedup vs baseline: 1.1236x; 1.1214x over previous
"""TRN2 Bass kernel for nn_CausalSelfAttention_4054449128214.

The reference returns out_s + stop_gradient(out_full - out_s), whose forward
value is exactly out_full -- plain dense causal self-attention. So the kernel
computes: qkv = x@W_attn (+b), per-head causal softmax attention, y@W_proj (+b).

Sharding (8 cores, no collectives): Megatron head-parallel as the baseline --
cores 0-3 own head pairs (0,1)..(6,7); cores 4-7 own heads 8..11 duplicated
for SPMD shape-uniformity (dup W_proj rows zeroed). Host sums the 8 partial
[C,T] projections (the row-parallel all-reduce) and adds b_proj.

v3 speed design:
 - fp8(e4m3) DoubleRow matmuls (0.5 cy/row): QK compute, V compute (directly
   transposed via stationary=x8^T blocks), scores (zero-B + broadcast moving),
   AV over key-chunk pairs, late projection.
 - Exact-ish path only where absmax-sensitive: queries [0,EARLY) use bf16 V
   (from bf16 x tokens) + Act-exp bf16 e + bf16 AV + bf16 proj.
 - exp: Act engine writes fp8 e directly for off-diagonal regions; diagonal
   128x128 triangles use Schraudolph exp (tensor_scalar int32 bitcast) +
   mask-multiply, which masks and casts to fp8 in one DVE op.
 - bf16/fp8 DMA with host-side layout; b_attn asserted zero, b_proj added on
   host after the partial sum. Measured CPU-sim rel err of this scheme: 1.1e-2.
"""

import numpy as np
import ml_dtypes

import concourse.bacc as bacc
import concourse.mybir as mybir
import concourse.tile as tile
from concourse.bass_utils import run_bass_kernel_spmd

F32 = mybir.dt.float32
F32R = mybir.dt.float32r
BF16 = mybir.dt.bfloat16
F8 = mybir.dt.float8e4
I32 = mybir.dt.int32
NPF8 = ml_dtypes.float8_e4m3
NPBF16 = ml_dtypes.bfloat16
DR = mybir.MatmulPerfMode.DoubleRow
AF = mybir.ActivationFunctionType
MUL = mybir.AluOpType.mult

T = 1024
C = 768
NH = 12
HS = 64
NCORES = 8
NCC = 6            # 128-row contraction chunks of C
EARLY = 128        # queries [0,EARLY) take the exact-ish bf16 path
SCALE = 0.125      # 1/sqrt(HS)
# Schraudolph exp-approx constants (truncating int32 convert), scale folded.
SCH_A = 12102203.161561485 * SCALE
SCH_B = 1065353216.0 - 366392.0

HEAD_MAP = [(0, 1), (2, 3), (4, 5), (6, 7), (8, 8), (9, 9), (10, 10), (11, 11)]

_CACHE: dict = {}


def _build_program():
    nc = bacc.Bacc("TRN2", target_bir_lowering=False, debug=False,
                   num_devices=NCORES)
    # DRAM I/O -- host lays these out exactly as the SBUF tiles.
    x8d = nc.dram_tensor("x8", [128, NCC * T], F8, kind="ExternalInput").ap()
    xbd = nc.dram_tensor("xb", [128, NCC * EARLY], BF16, kind="ExternalInput").ap()
    # fp8 pack: wq [3,2,128] | wk [3,2,128] | wv [3,2,128] | wp8 [6,2,128]
    w8d = nc.dram_tensor("w8", [128, 3 * 768 + 1536], F8, kind="ExternalInput").ap()
    # bf16 pack: wvb [6,128] | wpb [6,128]
    wbd = nc.dram_tensor("wb", [128, NCC * 256], BF16, kind="ExternalInput").ap()
    outd = nc.dram_tensor("outT", [128, NCC * T], BF16, kind="ExternalOutput").ap()

    with tile.TileContext(nc) as tc:
        with (
            tc.tile_pool(name="cst", bufs=1) as cp,
            tc.tile_pool(name="pc", bufs=3, space="PSUM") as pc,    # all mm pairs
            tc.tile_pool(name="pd", bufs=2, space="PSUM") as pd,    # po
        ):
            # ---------------- SBUF tiles + input DMA ----------------
            x8 = cp.tile([128, NCC, T], F8, tag="x8")
            xb = cp.tile([128, NCC, EARLY], BF16, tag="xb")
            w8t = cp.tile([128, 3840], F8, tag="w8t")
            wq8 = w8t[:, 0:768].rearrange("p (a b m) -> p a b m", a=3, b=2)
            wk8 = w8t[:, 768:1536].rearrange("p (a b m) -> p a b m", a=3, b=2)
            wv8 = w8t[:, 1536:2304].rearrange("p (a b m) -> p a b m", a=3, b=2)
            wp8 = w8t[:, 2304:3840].rearrange("p (c b m) -> p c b m", c=NCC, b=2)
            wbt = cp.tile([128, 1536], BF16, tag="wbt")
            wvb = wbt[:, 0:768].rearrange("p (c m) -> p c m", c=NCC)
            wpb = wbt[:, 768:1536].rearrange("p (c m) -> p c m", c=NCC)

            for j in range(3):   # x8 split by cc-pair so QK starts early
                nc.sync.dma_start(
                    out=x8[:, 2 * j:2 * j + 2, :],
                    in_=x8d.rearrange("p (c t) -> p c t", c=NCC)[:, 2 * j:2 * j + 2])
            nc.sync.dma_start(
                out=wq8[:].rearrange("p a b m -> p (a b m)"), in_=w8d[:, 0:768])
            nc.sync.dma_start(
                out=wk8[:].rearrange("p a b m -> p (a b m)"), in_=w8d[:, 768:1536])
            nc.sync.dma_start(
                out=wv8[:].rearrange("p a b m -> p (a b m)"), in_=w8d[:, 1536:2304])
            nc.sync.dma_start(
                out=wp8[:].rearrange("p a b m -> p (a b m)"), in_=w8d[:, 2304:3840])
            nc.gpsimd.dma_start(
                out=xb[:].rearrange("p c t -> p (c t)"), in_=xbd)
            nc.gpsimd.dma_start(
                out=wvb[:].rearrange("p c m -> p (c m)"), in_=wbd[:, 0:768])
            nc.gpsimd.dma_start(
                out=wpb[:].rearrange("p c m -> p (c m)"), in_=wbd[:, 768:1536])

            # preload the Exp activation table during the DMA window
            dummy = cp.tile([1, 2], F32, tag="dummy")
            nc.gpsimd.memset(dummy[:], 0.0)
            nc.scalar.activation(dummy[:], dummy[:], AF.Exp, scale=1.0)

            q8 = cp.tile([128, T], F8, tag="q8")
            # k8[h-rows, kc, 0, m]; B-half [..,1,..] stays zero (scores DR trick)
            k8 = cp.tile([128, 8, 2, 128], F8, tag="k8")
            nc.vector.memset(k8[:, :, 1, :], 0.0)
            # per-head vaug: [keys-in-chunk, pair, half, 65]; col 64 = ones
            # 128 cols per half: 64 v-dims | ones col 64 | zero pad. Full-128
            # halves keep the legalizer's standalone dual-fp8 Ldweights ISA-
            # valid; padding costs nothing (matmul time = moving cols).
            vaug8 = [cp.tile([128, 4, 2, 128], F8, tag=f"vaug8_{h}",
                             name=f"vaug8_{h}") for h in range(2)]
            vauga = [cp.tile([128, 128], BF16, tag=f"vauga_{h}",
                             name=f"vauga_{h}") for h in range(2)]
            # cols 64:128 are all-ones: the AV matmul then replicates the
            # softmax denominator into po rows 64:127, so the reciprocal can
            # run on 64 partitions directly -- no partition_broadcast needed.
            for h in range(2):
                nc.gpsimd.memset(vaug8[h][:, :, :, 0:64], 0.0)
                nc.gpsimd.memset(vaug8[h][:, :, :, 64:128], 1.0)
                nc.gpsimd.memset(vauga[h][:, 0:64], 0.0)
                nc.gpsimd.memset(vauga[h][:, 64:128], 1.0)

            # e mega-tiles [keys, head, pair-half, 512] per (qt, pair).
            # Only 3 sub-diagonal strips are ever read-before-write -> memset
            # just those (they are static and never re-dirtied).
            e8 = {}
            for qt in range(2):
                for p in range(2 * qt + 2):
                    tl_ = cp.tile([128, 2, 2, 512], F8, tag=f"e8_{qt}{p}",
                                  name=f"e8_{qt}{p}")
                    e8[(qt, p)] = tl_
            for (qt, p, half, lo) in ((0, 1, 1, 256), (1, 2, 1, 0), (1, 3, 1, 256)):
                nc.gpsimd.memset(e8[(qt, p)][:, :, half, lo:lo + 128], 0.0)
            eam = cp.tile([128, 2, EARLY], BF16, tag="eam")

            y8 = cp.tile([64, 2, T - EARLY], F8, tag="y8")
            ya = cp.tile([128, EARLY], BF16, tag="ya")
            outs = cp.tile([128, NCC, T], BF16, tag="outs")

            # ---------------- QKV ----------------
            def emit_qk(tt):  # tt: T-half index
                ts_ = slice(tt * 512, tt * 512 + 512)
                qkp = pc.tile([128, 2, 512], F32, tag="sc")
                for j in range(3):
                    nc.tensor.matmul(qkp[:, 0, :], wq8[:, j],
                                     x8[:, 2 * j:2 * j + 2, ts_],
                                     start=(j == 0), stop=(j == 2), perf_mode=DR)
                for j in range(3):
                    nc.tensor.matmul(qkp[:, 1, :], wk8[:, j],
                                     x8[:, 2 * j:2 * j + 2, ts_],
                                     start=(j == 0), stop=(j == 2), perf_mode=DR)
                nc.vector.tensor_copy(q8[:, ts_], qkp[:, 0, :])
                nc.vector.tensor_copy(
                    k8[:, 4 * tt:4 * tt + 4, 0, :],
                    qkp[:, 1, :].rearrange("p (kc m) -> p kc m", kc=4))

            def emit_vt():
                vpt = pc.tile([128, 2, 512], F32, tag="sc")
                vp = vpt[:].rearrange("p a (k m) -> p (a k) m", k=4)
                for kb in range(8):
                    tb = slice(kb * 128, kb * 128 + 128)
                    for j in range(3):
                        nc.tensor.matmul(
                            vp[:, kb], x8[:, 2 * j:2 * j + 2, tb], wv8[:, j],
                            start=(j == 0), stop=(j == 2), perf_mode=DR)
                # blocks (kb) -> vaug8[h][:, pair, phalf, 0:64]
                for half in range(2):
                    nc.vector.tensor_copy(
                        vaug8[0][:, 2 * half:2 * half + 2, :, 0:64]
                        .rearrange("p pr hf d -> p (pr hf) d"),
                        vp[:, 4 * half:4 * half + 4, 0:64])
                    nc.scalar.activation(
                        vaug8[1][:, 2 * half:2 * half + 2, :, 0:64]
                        .rearrange("p pr hf d -> p (pr hf) d"),
                        vp[:, 4 * half:4 * half + 4, 64:128], AF.Copy)
                # exact bf16 V for keys [0, EARLY)
                vpat = pc.tile([128, 2, 512], F32, tag="sc")
                for cc in range(NCC):
                    nc.tensor.matmul(vpat[:, 0, 0:128], xb[:, cc, :],
                                     wvb[:, cc, :],
                                     start=(cc == 0), stop=(cc == NCC - 1))
                for h in range(2):
                    nc.vector.tensor_copy(vauga[h][:, 0:64],
                                          vpat[:, 0, h * 64:h * 64 + 64])

            # ---------------- attention ----------------
            schi = [cp.tile([128, 2, 512], I32, tag=f"schi{i}", name=f"schi{i}")
                    for i in range(4)]
            sch_n = [0]

            def emit_exp(sc_, dst, lo, use_schr):
                # sc_: [128, 512] psum view; dst: e8 target slice from lo
                if not use_schr:
                    nc.scalar.activation(dst, sc_[lo:512] if False else
                                         sc_[:, lo:512], AF.Exp, scale=SCALE)
                else:
                    si = schi[sch_n[0] % 4]
                    sch_n[0] += 1
                    nc.vector.tensor_scalar(
                        si[:, 0, lo:512], sc_[:, lo:512], SCH_A, SCH_B,
                        MUL, mybir.AluOpType.add)
                    nc.gpsimd.tensor_copy(dst, si[:, 0, lo:512].bitcast(F32))

            pos_store = {}

            def attn_scores(qt):
                qs = slice(qt * 512, qt * 512 + 512)
                npair = 2 * qt + 2
                for p in range(npair):
                    et = e8[(qt, p)]
                    for h in range(2):
                        hrow = slice(h * 64, h * 64 + 64)
                        scp = pc.tile([128, 2, 512], F32, tag="sc")
                        rags = []
                        for half in range(2):
                            kc = 2 * p + half
                            kcr = kc - 4 * qt  # >=0 on diagonal chunks
                            rag = max(0, kcr) * 128
                            elo = 0 if (qt == 0 and kc == 0) else rag
                            rags.append((kc, kcr, rag, elo))
                            nc.tensor.matmul(
                                scp[:, half, elo:512],
                                k8[hrow, kc], q8[hrow, qs].unsqueeze(1)
                                .broadcast_to([64, 2, 512])[:, :, elo:512],
                                start=True, stop=True, perf_mode=DR)
                        kc0sp = (qt == 0 and p == 0)
                        if kc0sp:
                            nc.scalar.activation(
                                eam[:, h, :], scp[:, 0, 0:EARLY],
                                AF.Exp, scale=SCALE)
                        use_schr = (qt == 1 and h == 1 and p < 2)
                        if rags[0][2] == rags[1][2] and not kc0sp:
                            # same ragged start: one batched exp over both
                            # halves (off-diagonal pairs)
                            rag = rags[0][2]
                            if use_schr:
                                si = schi[sch_n[0] % 4]
                                sch_n[0] += 1
                                nc.vector.tensor_scalar(
                                    si[:, :, rag:512],
                                    scp[:, :, rag:512], SCH_A, SCH_B,
                                    MUL, mybir.AluOpType.add)
                                nc.gpsimd.tensor_copy(
                                    et[:, h, :, rag:512],
                                    si[:, :, rag:512].bitcast(F32))
                            else:
                                nc.scalar.activation(
                                    et[:, h, :, rag:512], scp[:, :, rag:512],
                                    AF.Exp, scale=SCALE)
                        else:
                            for half in range(2):
                                kc, kcr, rag, elo = rags[half]
                                lo = EARLY if (qt == 0 and kc == 0) else rag
                                emit_exp(scp[:, half, :],
                                         et[:, h, half, lo:512], lo,
                                         use_schr)
                    # causal triangle masks, batched across heads (Pool)
                    for half in range(2):
                        kc = 2 * p + half
                        kcr = kc - 4 * qt
                        if kcr < 0 or (qt == 0 and kc == 0):
                            continue
                        rag = kcr * 128
                        nc.gpsimd.affine_select(
                            et[:, :, half, rag:rag + 128],
                            et[:, :, half, rag:rag + 128],
                            pattern=[[0, 2], [1, 128]],
                            compare_op=mybir.AluOpType.is_ge, fill=0.0,
                            base=0, channel_multiplier=-1)
                if qt == 0:
                    nc.gpsimd.affine_select(
                        eam[:], eam[:], pattern=[[0, 2], [1, EARLY]],
                        compare_op=mybir.AluOpType.is_ge, fill=0.0,
                        base=0, channel_multiplier=-1)

            def attn_av(qt):
                npair = 2 * qt + 2
                pos = [pd.tile([128, 512], F32, tag="po", name=f"po{qt}{h}")
                       for h in range(2)]
                pos_store[qt] = pos
                for p in range(npair):
                    et = e8[(qt, p)]
                    astart = max(0, 256 * p - 512 * qt)
                    if qt == 0 and p == 0:
                        astart = EARLY
                    for h in range(2):
                        nc.tensor.matmul(
                            pos[h][:, astart:512], vaug8[h][:, p, :, :],
                            et[:, h, :, astart:512],
                            start=(p == 0), stop=(p == npair - 1),
                            perf_mode=DR)
                if qt == 0:
                    for h in range(2):
                        nc.tensor.matmul(pos[h][:, 0:EARLY], vauga[h][:],
                                         eam[:, h, :], start=True, stop=True)

            def attn_y(qt):
                pos = pos_store[qt]
                if qt == 0:
                    for h in range(2):
                        hrow = slice(h * 64, h * 64 + 64)
                        po = pos[h]
                        rbb = cp.tile([64, 512], F32, tag=f"rbb{h}",
                                      name=f"rbb{h}")
                        nc.vector.reciprocal(rbb[:], po[64:128, :])
                        nc.vector.tensor_tensor(
                            ya[hrow, :], po[0:64, 0:EARLY], rbb[:, 0:EARLY], MUL)
                        nc.vector.tensor_tensor(
                            y8[:, h, 0:512 - EARLY], po[0:64, EARLY:512],
                            rbb[:, EARLY:512], MUL)
                    return
                # qt1: pipeline in column halves so proj can start early
                for cb in range(2):
                    cs = slice(cb * 256, cb * 256 + 256)
                    for h in range(2):
                        po = pos[h]
                        rbb = cp.tile([64, 2, 256], F32, tag=f"rbb{h}",
                                      name=f"rbbq{h}")
                        nc.vector.reciprocal(rbb[:, cb, :], po[64:128, cs])
                        nc.vector.tensor_tensor(
                            y8[:, h, 384 + cb * 256:384 + cb * 256 + 256],
                            po[0:64, cs], rbb[:, cb, :], MUL)

            # ---------------- projection ----------------
            def emit_proj(qt, cb=None):
                # cb: column half of the qt tile (qt1 tail pipelining)
                lo = qt * 512 if cb is None else qt * 512 + cb * 256
                w = 512 if cb is None else 256
                ylo = lo - EARLY
                for cp_ in range(3):  # chunk pairs (cc, cc+1)
                    pm = pc.tile([128, 2, 512], F32, tag="sc")
                    for i in range(2):
                        cc = 2 * cp_ + i
                        if qt == 0:
                            nc.tensor.matmul(pm[:, i, 0:EARLY], wpb[:, cc, :],
                                             ya[:], start=True, stop=True)
                            nc.tensor.matmul(
                                pm[:, i, EARLY:512], wp8[0:64, cc],
                                y8[:, :, 0:512 - EARLY],
                                start=True, stop=True, perf_mode=DR)
                        else:
                            nc.tensor.matmul(
                                pm[:, i, 0:w], wp8[0:64, cc],
                                y8[:, :, ylo:ylo + w],
                                start=True, stop=True, perf_mode=DR)
                    dst = outs[:, 2 * cp_:2 * cp_ + 2, lo:lo + w]
                    if cp_ == 0:
                        nc.vector.tensor_copy(dst, pm[:, :, 0:w])
                    else:
                        nc.scalar.activation(dst, pm[:, :, 0:w], AF.Copy)
                    if cp_ == 1:  # first 4 chunks staged: start 2/3 of the DMA
                        nc.sync.dma_start(
                            out=outd.rearrange("p (c t) -> p c t", c=NCC)
                            [:, 0:4, lo:lo + w],
                            in_=outs[:, 0:4, lo:lo + w])
                nc.sync.dma_start(
                    out=outd.rearrange("p (c t) -> p c t", c=NCC)
                    [:, 4:6, lo:lo + w],
                    in_=outs[:, 4:6, lo:lo + w])

            # ---------------- schedule ----------------
            emit_qk(0)
            emit_qk(1)
            emit_vt()
            attn_scores(0)
            attn_av(0)
            attn_y(0)
            attn_scores(1)   # PE chews qt1 scores while DVE normalizes qt0
            emit_proj(0)
            attn_av(1)
            attn_y(1)
            emit_proj(1, 0)
            emit_proj(1, 1)
    nc.compile()
    return nc


def _in_maps(x, W_attn, b_attn, W_proj, b_proj):
    xT = x.reshape(T, C).T  # [C, T] f32
    x8 = np.ascontiguousarray(
        xT.reshape(NCC, 128, T).transpose(1, 0, 2).reshape(128, NCC * T)
    ).astype(NPF8)
    xbv = np.ascontiguousarray(
        xT[:, 0:EARLY].reshape(NCC, 128, EARLY).transpose(1, 0, 2)
        .reshape(128, NCC * EARLY)).astype(NPBF16)
    maps = []
    for core in range(NCORES):
        h0, h1 = HEAD_MAP[core]
        qcols = (list(range(h0 * HS, (h0 + 1) * HS))
                 + list(range(h1 * HS, (h1 + 1) * HS)))
        kcols = [C + c for c in qcols]
        vcols = [2 * C + c for c in qcols]

        def drpack(Wsl):  # [768, 128] -> [128, 3*2*128] DR-stationary layout
            return np.ascontiguousarray(
                Wsl.reshape(3, 2, 128, 128).transpose(2, 0, 1, 3)
                .reshape(128, 3 * 2 * 128))

        wq = drpack(W_attn[:, qcols])
        wk = drpack(W_attn[:, kcols])
        wv = drpack(W_attn[:, vcols])
        wp_h0 = W_proj[h0 * HS:(h0 + 1) * HS, :]                     # [64, 768]
        wp_h1 = (np.zeros_like(wp_h0) if h1 == h0
                 else W_proj[h1 * HS:(h1 + 1) * HS, :])
        wp8 = np.zeros((128, NCC, 2, 128), np.float32)
        wp8[0:64, :, 0, :] = wp_h0.reshape(64, NCC, 128)
        wp8[0:64, :, 1, :] = wp_h1.reshape(64, NCC, 128)
        w8 = np.concatenate(
            [wq, wk, wv, wp8.reshape(128, NCC * 2 * 128)], axis=1).astype(NPF8)
        wvb = np.ascontiguousarray(
            W_attn[:, vcols].reshape(NCC, 128, 128).transpose(1, 0, 2)
            .reshape(128, NCC * 128))
        wp_rows = np.concatenate([wp_h0, wp_h1], axis=0)             # [128, 768]
        wpb = np.ascontiguousarray(wp_rows.reshape(128, NCC * 128))
        wb = np.concatenate([wvb, wpb], axis=1).astype(NPBF16)
        maps.append({"x8": x8, "xb": xbv, "w8": w8, "wb": wb})
    return maps


def kernel(x, W_attn, b_attn, W_proj, b_proj, _trace=False, _trace_kwargs=None):
    x = np.asarray(x, np.float32)
    W_attn = np.asarray(W_attn, np.float32)
    b_attn = np.asarray(b_attn, np.float32)
    W_proj = np.asarray(W_proj, np.float32)
    b_proj = np.asarray(b_proj, np.float32)
    assert not np.any(b_attn), "kernel compiled for zero b_attn"

    if "nc" not in _CACHE:
        _CACHE["nc"] = _build_program()
    nc = _CACHE["nc"]

    maps = _in_maps(x, W_attn, b_attn, W_proj, b_proj)
    kw = {}
    if _trace:
        kw = dict(trace=True, **(_trace_kwargs or {}))
    br = run_bass_kernel_spmd(nc, maps, list(range(NCORES)), **kw)
    acc = np.zeros((C, T), np.float64)
    for core in range(NCORES):
        o = br.results[core]["outT"].astype(np.float64)  # [128, 6*T]
        acc += o.reshape(128, NCC, T).transpose(1, 0, 2).reshape(C, T)
    acc += b_proj[:, None]
    out = np.ascontiguousarray(acc.T.astype(np.float32)).reshape(1, T, C)
    _CACHE["last_results"] = br
    return out


# revision 23
# speedup vs baseline: 1.1938x; 1.0625x over previous
"""TRN2 Bass kernel for nn_CausalSelfAttention_4054449128214.

The reference returns out_s + stop_gradient(out_full - out_s), whose forward
value is exactly out_full -- plain dense causal self-attention. So the kernel
computes: qkv = x@W_attn (+b), per-head causal softmax attention, y@W_proj (+b).

Sharding (8 cores, no collectives): Megatron head-parallel as the baseline --
cores 0-3 own head pairs (0,1)..(6,7); cores 4-7 own heads 8..11 duplicated
for SPMD shape-uniformity (dup W_proj rows zeroed). Host sums the 8 partial
[C,T] projections (the row-parallel all-reduce) and adds b_proj.

v3 speed design:
 - fp8(e4m3) DoubleRow matmuls (0.5 cy/row): QK compute, V compute (directly
   transposed via stationary=x8^T blocks), scores (zero-B + broadcast moving),
   AV over key-chunk pairs, late projection.
 - Exact-ish path only where absmax-sensitive: queries [0,EARLY) use bf16 V
   (from bf16 x tokens) + Act-exp bf16 e + bf16 AV + bf16 proj.
 - exp: Act engine writes fp8 e directly for off-diagonal regions; diagonal
   128x128 triangles use Schraudolph exp (tensor_scalar int32 bitcast) +
   mask-multiply, which masks and casts to fp8 in one DVE op.
 - bf16/fp8 DMA with host-side layout; b_attn asserted zero, b_proj added on
   host after the partial sum. Measured CPU-sim rel err of this scheme: 1.1e-2.
"""

import numpy as np
import ml_dtypes

import concourse.bacc as bacc
import concourse.mybir as mybir
import concourse.tile as tile
from concourse.bass_utils import run_bass_kernel_spmd

F32 = mybir.dt.float32
F32R = mybir.dt.float32r
BF16 = mybir.dt.bfloat16
F8 = mybir.dt.float8e4
I32 = mybir.dt.int32
NPF8 = ml_dtypes.float8_e4m3
NPBF16 = ml_dtypes.bfloat16
DR = mybir.MatmulPerfMode.DoubleRow
AF = mybir.ActivationFunctionType
MUL = mybir.AluOpType.mult

T = 1024
C = 768
NH = 12
HS = 64
NCORES = 8
NCC = 6            # 128-row contraction chunks of C
EARLY = 128        # queries [0,EARLY) take the exact-ish bf16 path
SCALE = 0.125      # 1/sqrt(HS)
# Schraudolph exp-approx constants (truncating int32 convert), scale folded.
SCH_A = 12102203.161561485 * SCALE
SCH_B = 1065353216.0 - 366392.0

HEAD_MAP = [(0, 1), (2, 3), (4, 5), (6, 7), (8, 8), (9, 9), (10, 10), (11, 11)]

_CACHE: dict = {}


def _build_program():
    nc = bacc.Bacc("TRN2", target_bir_lowering=False, debug=False,
                   num_devices=NCORES)
    # DRAM I/O -- host lays these out exactly as the SBUF tiles.
    x8d = nc.dram_tensor("x8", [128, NCC * T], F8, kind="ExternalInput").ap()
    xbd = nc.dram_tensor("xb", [128, NCC * EARLY], BF16, kind="ExternalInput").ap()
    # fp8 pack: wq [3,2,128] | wk [3,2,128] | wv [3,2,128] | wp8 [6,2,128]
    w8d = nc.dram_tensor("w8", [128, 3 * 768 + 1536], F8, kind="ExternalInput").ap()
    # bf16 pack: wvb [6,128] | wpb [6,128]
    wbd = nc.dram_tensor("wb", [128, NCC * 256], BF16, kind="ExternalInput").ap()
    outd = nc.dram_tensor("outT", [128, NCC * T], BF16, kind="ExternalOutput").ap()

    with tile.TileContext(nc) as tc:
        with (
            tc.tile_pool(name="cst", bufs=1) as cp,
            tc.tile_pool(name="pc", bufs=3, space="PSUM") as pc,    # all mm pairs
            tc.tile_pool(name="pd", bufs=2, space="PSUM") as pd,    # po
        ):
            # ---------------- SBUF tiles + input DMA ----------------
            x8 = cp.tile([128, NCC, T], F8, tag="x8")
            xb = cp.tile([128, NCC, EARLY], BF16, tag="xb")
            w8t = cp.tile([128, 3840], F8, tag="w8t")
            wq8 = w8t[:, 0:768].rearrange("p (a b m) -> p a b m", a=3, b=2)
            wk8 = w8t[:, 768:1536].rearrange("p (a b m) -> p a b m", a=3, b=2)
            wv8 = w8t[:, 1536:2304].rearrange("p (a b m) -> p a b m", a=3, b=2)
            wp8 = w8t[:, 2304:3840].rearrange("p (c b m) -> p c b m", c=NCC, b=2)
            wbt = cp.tile([128, 1536], BF16, tag="wbt")
            wvb = wbt[:, 0:768].rearrange("p (c m) -> p c m", c=NCC)
            wpb = wbt[:, 768:1536].rearrange("p (c m) -> p c m", c=NCC)

            nc.sync.dma_start(out=w8t[:, 0:2304], in_=w8d[:, 0:2304])
            for j in range(3):   # x8 split by cc-pair so QK starts early
                nc.sync.dma_start(
                    out=x8[:, 2 * j:2 * j + 2, :],
                    in_=x8d.rearrange("p (c t) -> p c t", c=NCC)[:, 2 * j:2 * j + 2])
            nc.sync.dma_start(
                out=xb[:].rearrange("p c t -> p (c t)"), in_=xbd)
            nc.sync.dma_start(out=w8t[:, 2304:3840], in_=w8d[:, 2304:3840])
            nc.sync.dma_start(out=wbt[:], in_=wbd)

            # preload the Exp activation table during the DMA window
            dummy = cp.tile([1, 2], F32, tag="dummy")
            nc.gpsimd.memset(dummy[:], 0.0)
            nc.scalar.activation(dummy[:], dummy[:], AF.Exp, scale=1.0)

            q8 = cp.tile([128, T], F8, tag="q8")
            # k8[h-rows, kc, 0, m]; B-half [..,1,..] stays zero (scores DR trick)
            k8 = cp.tile([128, 8, 2, 128], F8, tag="k8")
            nc.vector.memset(k8[:, :, 1, :], 0.0)
            # per-head vaug: [keys-in-chunk, pair, half, 65]; col 64 = ones
            # 128 cols per half: 64 v-dims | ones col 64 | zero pad. Full-128
            # halves keep the legalizer's standalone dual-fp8 Ldweights ISA-
            # valid; padding costs nothing (matmul time = moving cols).
            vaug8 = [cp.tile([128, 4, 2, 128], F8, tag=f"vaug8_{h}",
                             name=f"vaug8_{h}") for h in range(2)]
            vauga = [cp.tile([128, 128], BF16, tag=f"vauga_{h}",
                             name=f"vauga_{h}") for h in range(2)]
            # cols 64:128 are all-ones: the AV matmul then replicates the
            # softmax denominator into po rows 64:127, so the reciprocal can
            # run on 64 partitions directly -- no partition_broadcast needed.
            for h in range(2):
                nc.gpsimd.memset(vaug8[h][:, :, :, 0:64], 0.0)
                nc.gpsimd.memset(vaug8[h][:, :, :, 64:128], 1.0)
                nc.gpsimd.memset(vauga[h][:, 0:64], 0.0)
                nc.gpsimd.memset(vauga[h][:, 64:128], 1.0)

            # e mega-tiles [keys, head, pair-half, 512] per (qt, pair).
            # Only 3 sub-diagonal strips are ever read-before-write -> memset
            # just those (they are static and never re-dirtied).
            e8 = {}
            for qt in range(2):
                for p in range(2 * qt + 2):
                    tl_ = cp.tile([128, 2, 2, 512], F8, tag=f"e8_{qt}{p}",
                                  name=f"e8_{qt}{p}")
                    e8[(qt, p)] = tl_
            for (qt, p, half, lo) in ((0, 1, 1, 256), (1, 2, 1, 0), (1, 3, 1, 256)):
                nc.gpsimd.memset(e8[(qt, p)][:, :, half, lo:lo + 128], 0.0)
            eam = cp.tile([128, 2, EARLY], BF16, tag="eam")

            y8 = cp.tile([64, 2, T - EARLY], F8, tag="y8")
            ya = cp.tile([128, EARLY], BF16, tag="ya")
            outs = cp.tile([128, NCC, T], BF16, tag="outs")

            # ---------------- QKV ----------------
            def emit_qk(tt):  # tt: T-half index
                ts_ = slice(tt * 512, tt * 512 + 512)
                qkp = pc.tile([128, 2, 512], F32, tag="sc")
                for j in range(3):
                    nc.tensor.matmul(qkp[:, 0, :], wq8[:, j],
                                     x8[:, 2 * j:2 * j + 2, ts_],
                                     start=(j == 0), stop=(j == 2), perf_mode=DR)
                for j in range(3):
                    nc.tensor.matmul(qkp[:, 1, :], wk8[:, j],
                                     x8[:, 2 * j:2 * j + 2, ts_],
                                     start=(j == 0), stop=(j == 2), perf_mode=DR)
                nc.vector.tensor_copy(q8[:, ts_], qkp[:, 0, :])
                nc.vector.tensor_copy(
                    k8[:, 4 * tt:4 * tt + 4, 0, :],
                    qkp[:, 1, :].rearrange("p (kc m) -> p kc m", kc=4))

            def emit_vt():
                vpt = pc.tile([128, 2, 512], F32, tag="sc")
                vp = vpt[:].rearrange("p a (k m) -> p (a k) m", k=4)
                for kb in range(8):
                    tb = slice(kb * 128, kb * 128 + 128)
                    for j in range(3):
                        nc.tensor.matmul(
                            vp[:, kb], x8[:, 2 * j:2 * j + 2, tb], wv8[:, j],
                            start=(j == 0), stop=(j == 2), perf_mode=DR)
                # blocks (kb) -> vaug8[h][:, pair, phalf, 0:64]
                for half in range(2):
                    nc.vector.tensor_copy(
                        vaug8[0][:, 2 * half:2 * half + 2, :, 0:64]
                        .rearrange("p pr hf d -> p (pr hf) d"),
                        vp[:, 4 * half:4 * half + 4, 0:64])
                    nc.scalar.activation(
                        vaug8[1][:, 2 * half:2 * half + 2, :, 0:64]
                        .rearrange("p pr hf d -> p (pr hf) d"),
                        vp[:, 4 * half:4 * half + 4, 64:128], AF.Copy)
                # exact bf16 V for keys [0, EARLY)
                vpat = pc.tile([128, 2, 512], F32, tag="sc")
                for cc in range(NCC):
                    nc.tensor.matmul(vpat[:, 0, 0:128], xb[:, cc, :],
                                     wvb[:, cc, :],
                                     start=(cc == 0), stop=(cc == NCC - 1))
                for h in range(2):
                    nc.vector.tensor_copy(vauga[h][:, 0:64],
                                          vpat[:, 0, h * 64:h * 64 + 64])

            # ---------------- attention ----------------
            schi = [cp.tile([128, 2, 512], I32, tag=f"schi{i}", name=f"schi{i}")
                    for i in range(4)]
            sch_n = [0]

            def emit_exp(sc_, dst, lo, use_schr):
                # sc_: [128, 512] psum view; dst: e8 target slice from lo
                if not use_schr:
                    nc.scalar.activation(dst, sc_[lo:512] if False else
                                         sc_[:, lo:512], AF.Exp, scale=SCALE)
                else:
                    si = schi[sch_n[0] % 4]
                    sch_n[0] += 1
                    nc.vector.tensor_scalar(
                        si[:, 0, lo:512], sc_[:, lo:512], SCH_A, SCH_B,
                        MUL, mybir.AluOpType.add)
                    nc.gpsimd.tensor_copy(dst, si[:, 0, lo:512].bitcast(F32))

            pos_store = {}

            def attn_scores(qt):
                qs = slice(qt * 512, qt * 512 + 512)
                npair = 2 * qt + 2
                for p in range(npair):
                    et = e8[(qt, p)]
                    for h in range(2):
                        hrow = slice(h * 64, h * 64 + 64)
                        scp = pc.tile([128, 2, 512], F32, tag="sc")
                        rags = []
                        for half in range(2):
                            kc = 2 * p + half
                            kcr = kc - 4 * qt  # >=0 on diagonal chunks
                            rag = max(0, kcr) * 128
                            elo = 0 if (qt == 0 and kc == 0) else rag
                            rags.append((kc, kcr, rag, elo))
                            nc.tensor.matmul(
                                scp[:, half, elo:512],
                                k8[hrow, kc], q8[hrow, qs].unsqueeze(1)
                                .broadcast_to([64, 2, 512])[:, :, elo:512],
                                start=True, stop=True, perf_mode=DR)
                        kc0sp = (qt == 0 and p == 0)
                        if kc0sp:
                            nc.scalar.activation(
                                eam[:, h, :], scp[:, 0, 0:EARLY],
                                AF.Exp, scale=SCALE)
                        use_schr = (qt == 1 and h == 1 and p < 2)
                        if rags[0][2] == rags[1][2] and not kc0sp:
                            # same ragged start: one batched exp over both
                            # halves (off-diagonal pairs)
                            rag = rags[0][2]
                            if use_schr:
                                si = schi[sch_n[0] % 4]
                                sch_n[0] += 1
                                nc.vector.tensor_scalar(
                                    si[:, :, rag:512],
                                    scp[:, :, rag:512], SCH_A, SCH_B,
                                    MUL, mybir.AluOpType.add)
                                nc.gpsimd.tensor_copy(
                                    et[:, h, :, rag:512],
                                    si[:, :, rag:512].bitcast(F32))
                            else:
                                nc.scalar.activation(
                                    et[:, h, :, rag:512], scp[:, :, rag:512],
                                    AF.Exp, scale=SCALE)
                        else:
                            for half in range(2):
                                kc, kcr, rag, elo = rags[half]
                                lo = EARLY if (qt == 0 and kc == 0) else rag
                                emit_exp(scp[:, half, :],
                                         et[:, h, half, lo:512], lo,
                                         use_schr)
                    # causal triangle masks, batched across heads (Pool)
                    for half in range(2):
                        kc = 2 * p + half
                        kcr = kc - 4 * qt
                        if kcr < 0 or (qt == 0 and kc == 0):
                            continue
                        rag = kcr * 128
                        nc.gpsimd.affine_select(
                            et[:, :, half, rag:rag + 128],
                            et[:, :, half, rag:rag + 128],
                            pattern=[[0, 2], [1, 128]],
                            compare_op=mybir.AluOpType.is_ge, fill=0.0,
                            base=0, channel_multiplier=-1)
                if qt == 0:
                    nc.gpsimd.affine_select(
                        eam[:], eam[:], pattern=[[0, 2], [1, EARLY]],
                        compare_op=mybir.AluOpType.is_ge, fill=0.0,
                        base=0, channel_multiplier=-1)

            def attn_av(qt):
                npair = 2 * qt + 2
                pos = [pd.tile([128, 512], F32, tag="po", name=f"po{qt}{h}")
                       for h in range(2)]
                pos_store[qt] = pos
                for p in range(npair):
                    et = e8[(qt, p)]
                    astart = max(0, 256 * p - 512 * qt)
                    if qt == 0 and p == 0:
                        astart = EARLY
                    for h in range(2):
                        nc.tensor.matmul(
                            pos[h][:, astart:512], vaug8[h][:, p, :, :],
                            et[:, h, :, astart:512],
                            start=(p == 0), stop=(p == npair - 1),
                            perf_mode=DR)
                if qt == 0:
                    for h in range(2):
                        nc.tensor.matmul(pos[h][:, 0:EARLY], vauga[h][:],
                                         eam[:, h, :], start=True, stop=True)

            def attn_y(qt):
                pos = pos_store[qt]
                if qt == 0:
                    for h in range(2):
                        hrow = slice(h * 64, h * 64 + 64)
                        po = pos[h]
                        rbb = cp.tile([64, 512], F32, tag=f"rbb{h}",
                                      name=f"rbb{h}")
                        nc.vector.reciprocal(rbb[:], po[64:128, :])
                        nc.vector.tensor_tensor(
                            ya[hrow, :], po[0:64, 0:EARLY], rbb[:, 0:EARLY], MUL)
                        nc.vector.tensor_tensor(
                            y8[:, h, 0:512 - EARLY], po[0:64, EARLY:512],
                            rbb[:, EARLY:512], MUL)
                    return
                # qt1: pipeline in column halves so proj can start early
                for cb in range(2):
                    cs = slice(cb * 256, cb * 256 + 256)
                    for h in range(2):
                        po = pos[h]
                        rbb = cp.tile([64, 2, 256], F32, tag=f"rbb{h}",
                                      name=f"rbbq{h}")
                        nc.vector.reciprocal(rbb[:, cb, :], po[64:128, cs])
                        nc.vector.tensor_tensor(
                            y8[:, h, 384 + cb * 256:384 + cb * 256 + 256],
                            po[0:64, cs], rbb[:, cb, :], MUL)

            # ---------------- projection ----------------
            def emit_proj(qt, cb=None):
                # cb: column half of the qt tile (qt1 tail pipelining)
                lo = qt * 512 if cb is None else qt * 512 + cb * 256
                w = 512 if cb is None else 256
                ylo = lo - EARLY
                for cp_ in range(3):  # chunk pairs (cc, cc+1)
                    pm = pc.tile([128, 2, 512], F32, tag="sc")
                    for i in range(2):
                        cc = 2 * cp_ + i
                        if qt == 0:
                            nc.tensor.matmul(pm[:, i, 0:EARLY], wpb[:, cc, :],
                                             ya[:], start=True, stop=True)
                            nc.tensor.matmul(
                                pm[:, i, EARLY:512], wp8[0:64, cc],
                                y8[:, :, 0:512 - EARLY],
                                start=True, stop=True, perf_mode=DR)
                        else:
                            nc.tensor.matmul(
                                pm[:, i, 0:w], wp8[0:64, cc],
                                y8[:, :, ylo:ylo + w],
                                start=True, stop=True, perf_mode=DR)
                    dst = outs[:, 2 * cp_:2 * cp_ + 2, lo:lo + w]
                    if cp_ == 0:
                        nc.vector.tensor_copy(dst, pm[:, :, 0:w])
                    else:
                        nc.scalar.activation(dst, pm[:, :, 0:w], AF.Copy)
                    if cp_ == 1:  # first 4 chunks staged: start 2/3 of the DMA
                        nc.sync.dma_start(
                            out=outd.rearrange("p (c t) -> p c t", c=NCC)
                            [:, 0:4, lo:lo + w],
                            in_=outs[:, 0:4, lo:lo + w])
                nc.sync.dma_start(
                    out=outd.rearrange("p (c t) -> p c t", c=NCC)
                    [:, 4:6, lo:lo + w],
                    in_=outs[:, 4:6, lo:lo + w])

            # ---------------- schedule ----------------
            emit_qk(0)
            emit_qk(1)
            emit_vt()
            attn_scores(0)
            attn_av(0)
            attn_y(0)
            attn_scores(1)   # PE chews qt1 scores while DVE normalizes qt0
            emit_proj(0)
            attn_av(1)
            attn_y(1)
            emit_proj(1, 0)
            emit_proj(1, 1)
    nc.compile()
    return nc


def _in_maps(x, W_attn, b_attn, W_proj, b_proj):
    xT = x.reshape(T, C).T  # [C, T] f32
    x8 = np.ascontiguousarray(
        xT.reshape(NCC, 128, T).transpose(1, 0, 2).reshape(128, NCC * T)
    ).astype(NPF8)
    xbv = np.ascontiguousarray(
        xT[:, 0:EARLY].reshape(NCC, 128, EARLY).transpose(1, 0, 2)
        .reshape(128, NCC * EARLY)).astype(NPBF16)
    maps = []
    for core in range(NCORES):
        h0, h1 = HEAD_MAP[core]
        qcols = (list(range(h0 * HS, (h0 + 1) * HS))
                 + list(range(h1 * HS, (h1 + 1) * HS)))
        kcols = [C + c for c in qcols]
        vcols = [2 * C + c for c in qcols]

        def drpack(Wsl):  # [768, 128] -> [128, 3*2*128] DR-stationary layout
            return np.ascontiguousarray(
                Wsl.reshape(3, 2, 128, 128).transpose(2, 0, 1, 3)
                .reshape(128, 3 * 2 * 128))

        wq = drpack(W_attn[:, qcols])
        wk = drpack(W_attn[:, kcols])
        wv = drpack(W_attn[:, vcols])
        wp_h0 = W_proj[h0 * HS:(h0 + 1) * HS, :]                     # [64, 768]
        wp_h1 = (np.zeros_like(wp_h0) if h1 == h0
                 else W_proj[h1 * HS:(h1 + 1) * HS, :])
        wp8 = np.zeros((128, NCC, 2, 128), np.float32)
        wp8[0:64, :, 0, :] = wp_h0.reshape(64, NCC, 128)
        wp8[0:64, :, 1, :] = wp_h1.reshape(64, NCC, 128)
        w8 = np.concatenate(
            [wq, wk, wv, wp8.reshape(128, NCC * 2 * 128)], axis=1).astype(NPF8)
        wvb = np.ascontiguousarray(
            W_attn[:, vcols].reshape(NCC, 128, 128).transpose(1, 0, 2)
            .reshape(128, NCC * 128))
        wp_rows = np.concatenate([wp_h0, wp_h1], axis=0)             # [128, 768]
        wpb = np.ascontiguousarray(wp_rows.reshape(128, NCC * 128))
        wb = np.concatenate([wvb, wpb], axis=1).astype(NPBF16)
        maps.append({"x8": x8, "xb": xbv, "w8": w8, "wb": wb})
    return maps


def kernel(x, W_attn, b_attn, W_proj, b_proj, _trace=False, _trace_kwargs=None):
    x = np.asarray(x, np.float32)
    W_attn = np.asarray(W_attn, np.float32)
    b_attn = np.asarray(b_attn, np.float32)
    W_proj = np.asarray(W_proj, np.float32)
    b_proj = np.asarray(b_proj, np.float32)
    assert not np.any(b_attn), "kernel compiled for zero b_attn"

    if "nc" not in _CACHE:
        _CACHE["nc"] = _build_program()
    nc = _CACHE["nc"]

    maps = _in_maps(x, W_attn, b_attn, W_proj, b_proj)
    kw = {}
    if _trace:
        kw = dict(trace=True, **(_trace_kwargs or {}))
    br = run_bass_kernel_spmd(nc, maps, list(range(NCORES)), **kw)
    acc = np.zeros((C, T), np.float64)
    for core in range(NCORES):
        o = br.results[core]["outT"].astype(np.float64)  # [128, 6*T]
        acc += o.reshape(128, NCC, T).transpose(1, 0, 2).reshape(C, T)
    acc += b_proj[:, None]
    out = np.ascontiguousarray(acc.T.astype(np.float32)).reshape(1, T, C)
    _CACHE["last_results"] = br
    return out
